# revision 4
# baseline (speedup 1.0000x reference)
"""Depthwise causal Conv1D (B=4, C=4096, L=4096, K=4) on 8 trn2 NeuronCores.

Sharding: channel-parallel (tensor parallel) — core i owns channels
[i*512, (i+1)*512). Depthwise conv has zero cross-channel interaction, so
there is no communication; each core computes its channel slab end to end.

The kernel is HBM-bandwidth bound, so all HBM I/O is bf16: x is converted
to bf16 on the host, streamed in at half the fp32 byte count, and the
output is stored bf16 and widened to fp32 on the host (the 2e-2 rel-err
budget dwarfs bf16's ~2^-9 rounding).

Per-core layout: channels on SBUF partitions (128 at a time), time on the
free dim. x lives in a [128, 3+L+3] tile with zero pads so out[m] =
sum_t w_t * xp[m+t]. The 4-tap FIR is split per 512-column chunk (PSUM
bank width) across three engines:

  PE      : taps 0,1,3 — diagonal-weight bf16 matmuls accumulating in
            PSUM (the odd-offset taps must avoid DVE: its 2x bf16
            packing needs 4B-aligned reads)
  ScalarE : out_bf16 = psum + bias   (activation, per-partition bias,
            reads PSUM, converts to bf16)
  VectorE : out_bf16 += w2 * xp[m+2] (scalar_tensor_tensor; +2 elements
            keeps the bf16 read 4B-aligned so the 2x mode engages)
  GpSimd  : zero-stuffs the x pads

Loads issue from sync's HWDGE, stores from ScalarE's (deferred one tile);
diag weight matrices are precomputed on the host.
"""

import numpy as np

import concourse.bass as bass
import concourse.tile as tile
from concourse import bacc, mybir
from concourse.bass_utils import run_bass_kernel_spmd

B, C, L, K = 4, 4096, 4096, 4
PAD = K - 1
LOUT = L + PAD  # 4099
NCORES = 8
CS = C // NCORES  # 512 channels per core
NG = CS // 128  # 4 partition groups per core
F32 = mybir.dt.float32
BF16 = mybir.dt.bfloat16

PE_TAPS = (0, 1, 3)  # taps accumulated on PE via diag matmuls
DVE_TAP = 2  # tap fused into the final DVE pass (4B-aligned in bf16)

_AF = mybir.ActivationFunctionType
_OP = mybir.AluOpType


def _chunks(l=L, lout=LOUT):
    """512-col chunks covering [0, lout): 7x512 then the 515 tail split
    258+257 so every chunk fits one PSUM bank and starts 4B-aligned."""
    out = []
    m0 = 0
    while lout - m0 > 515:
        out.append((m0, 512))
        m0 += 512
    rest = lout - m0
    a = (rest + 1) // 2
    a += a % 2  # keep the second chunk's start even (4B-aligned bf16)
    out.append((m0, a))
    out.append((m0 + a, rest - a))
    return out


def build_nc(b=B, cs=CS, l=L, k=K, n_bufs=8, n_load_chunks=2):
    ng = cs // 128
    pad = k - 1
    lout = l + pad
    wx = l + 2 * pad  # padded x width
    nt = len(PE_TAPS)

    nc = bacc.Bacc("TRN2", target_bir_lowering=False, debug=False, num_devices=NCORES)
    x_d = nc.dram_tensor("x", [b, cs, l], BF16, kind="ExternalInput").ap()
    # all ng*nt diagonal weight matrices packed side by side: one DMA,
    # full 3 KB partition rows (128x128 singles would move 256 B packets)
    dg_d = nc.dram_tensor("dg", [128, ng * nt * 128], BF16,
                          kind="ExternalInput").ap()
    ctw_d = nc.dram_tensor("ctw", [128, ng], BF16, kind="ExternalInput").ap()
    ctb_d = nc.dram_tensor("ctb", [128, ng], F32, kind="ExternalInput").ap()
    o_d = nc.dram_tensor("out", [b, cs, lout], BF16, kind="ExternalOutput").ap()

    chunks = _chunks(l, lout)

    with tile.TileContext(nc) as tc:
        with (
            tc.tile_pool(name="consts", bufs=1) as cpool,
            tc.tile_pool(name="xs", bufs=n_bufs) as xpool,
            tc.tile_pool(name="os", bufs=n_bufs) as opool,
            tc.tile_pool(name="ps", bufs=8, space="PSUM") as ppool,
        ):
            dgp = cpool.tile([128, ng * nt * 128], BF16, tag="dg")
            ctw = cpool.tile([128, ng], BF16, tag="cw")
            ctb = cpool.tile([128, ng], F32, tag="cb")

            def emit_consts():
                nc.sync.dma_start(ctw[:], ctw_d[:])
                nc.sync.dma_start(ctb[:], ctb_d[:])
                nc.sync.dma_start(dgp[:], dg_d[:])

            def diag(g, j):
                o = (g * nt + j) * 128
                return dgp[:, o : o + 128]

            n_tiles = b * ng
            # loads and stores alternate between the two HWDGE rings
            # (sync / scalar) per tile so both rings stream both
            # directions; a single ring in one direction caps ~190 GB/s
            # (write-receipt latency) while two mixed rings reach ~420.
            pending_stores = []  # deferred one tile to keep ring heads unblocked

            def flush_stores():
                for q, dst, src in pending_stores:
                    q.dma_start(dst, src)
                pending_stores.clear()

            ti = 0
            for bi in range(b):
                for g in range(ng):
                    c0 = g * 128
                    first, last = ti == 0, ti == n_tiles - 1
                    ldq = nc.sync if ti % 2 == 0 else nc.scalar
                    stq = nc.scalar if ti % 2 == 0 else nc.sync

                    xt = xpool.tile([128, wx], BF16, tag="x")
                    nc.gpsimd.memset(xt[:, 0:pad], 0.0)
                    nc.gpsimd.memset(xt[:, pad + l : wx], 0.0)
                    if first:
                        # chunked load so compute ramps before the full
                        # tile lands; consts follow the first chunk
                        cw = l // n_load_chunks
                        ldq.dma_start(
                            xt[:, pad : pad + cw], x_d[bi, c0 : c0 + 128, 0:cw]
                        )
                        emit_consts()
                        for c in range(1, n_load_chunks):
                            ldq.dma_start(
                                xt[:, pad + c * cw : pad + (c + 1) * cw],
                                x_d[bi, c0 : c0 + 128, c * cw : (c + 1) * cw],
                            )
                    else:
                        ldq.dma_start(
                            xt[:, pad : pad + l], x_d[bi, c0 : c0 + 128, :]
                        )
                    ot = opool.tile([128, lout], BF16, tag="o")

                    for ci, (m0, fd) in enumerate(chunks):
                        pt = ppool.tile([128, 512], F32, tag="p")
                        for j in range(nt):
                            nc.tensor.matmul(
                                pt[:, 0:fd], lhsT=diag(g, j),
                                rhs=xt[:, m0 + PE_TAPS[j] : m0 + PE_TAPS[j] + fd],
                                start=(j == 0), stop=(j == nt - 1),
                            )
                        # out = psum + bias (converts to bf16)
                        nc.scalar.activation(
                            ot[:, m0 : m0 + fd], pt[:, 0:fd], _AF.Identity,
                            bias=ctb[:, g : g + 1], scale=1.0,
                        )
                        if m0 == 0:
                            flush_stores()
                        # out += w2 * xp[m+2]
                        nc.vector.scalar_tensor_tensor(
                            out=ot[:, m0 : m0 + fd],
                            in0=xt[:, m0 + DVE_TAP : m0 + DVE_TAP + fd],
                            scalar=ctw[:, g : g + 1],
                            in1=ot[:, m0 : m0 + fd],
                            op0=_OP.mult, op1=_OP.add,
                        )
                        if last:
                            # drain: alternate the final chunk stores too
                            q = nc.scalar if ci % 2 == 0 else nc.sync
                            q.dma_start(
                                o_d[bi, c0 : c0 + 128, m0 : m0 + fd],
                                ot[:, m0 : m0 + fd],
                            )
                    if not last:
                        pending_stores.append(
                            (stq, o_d[bi, c0 : c0 + 128, :], ot[:])
                        )
                    ti += 1
            flush_stores()
    nc.compile()
    return nc


_cached_nc = None


def _get_nc():
    global _cached_nc
    if _cached_nc is None:
        _cached_nc = build_nc()
    return _cached_nc


def run(x, kernel, bias, trace=False, **kwargs):
    """Shard, run on 8 cores, gather. Returns (out, BassKernelResults)."""
    import ml_dtypes

    bf16 = ml_dtypes.bfloat16
    x_bf = np.ascontiguousarray(np.asarray(x, dtype=np.float32)).astype(bf16)
    w = np.asarray(kernel, dtype=np.float32).reshape(K, C)
    bvec = np.asarray(bias, dtype=np.float32).reshape(C)

    w_bf = w.astype(bf16)
    nt = len(PE_TAPS)
    in_maps = []
    for i in range(NCORES):
        sl = slice(i * CS, (i + 1) * CS)
        dg = np.zeros((NG * nt, 128, 128), dtype=bf16)
        ctw = np.zeros((128, NG), dtype=bf16)
        ctb = np.zeros((128, NG), dtype=np.float32)
        for g in range(NG):
            cg = slice(i * CS + g * 128, i * CS + (g + 1) * 128)
            for j, t in enumerate(PE_TAPS):
                np.fill_diagonal(dg[g * nt + j], w_bf[t, cg])
            ctw[:, g] = w_bf[DVE_TAP, cg]
            ctb[:, g] = bvec[cg]
        # pack [ng*nt, 128, 128] -> [128, ng*nt*128] partition-major
        dg_pack = np.ascontiguousarray(
            dg.transpose(1, 0, 2).reshape(128, NG * nt * 128)
        )
        in_maps.append(
            {
                "x": np.ascontiguousarray(x_bf[:, sl, :]),
                "dg": dg_pack,
                "ctw": ctw,
                "ctb": ctb,
            }
        )

    nc = _get_nc()
    bkr = run_bass_kernel_spmd(
        nc, in_maps, core_ids=list(range(NCORES)), trace=trace, **kwargs
    )
    out = np.concatenate(
        [r["out"].astype(np.float32) for r in bkr.results], axis=1
    )
    return out, bkr


def kernel(x, kernel, bias):
    import os

    prev = os.environ.get("BASS_NEVER_TRACE")
    os.environ["BASS_NEVER_TRACE"] = "1"  # keep the runner off the NTFF path
    try:
        out, _ = run(x, kernel, bias)
    finally:
        if prev is None:
            os.environ.pop("BASS_NEVER_TRACE", None)
        else:
            os.environ["BASS_NEVER_TRACE"] = prev
    return out


# revision 5
# speedup vs baseline: 1.2482x; 1.2482x over previous
"""Depthwise causal Conv1D (B=4, C=4096, L=4096, K=4) on 8 trn2 NeuronCores.

Sharding: channel-parallel (tensor parallel) — core i owns channels
[i*512, (i+1)*512). Depthwise conv has zero cross-channel interaction, so
there is no communication; each core computes its channel slab end to end.

The kernel is HBM-bandwidth bound, so all HBM I/O is bf16 (the 2e-2
rel-err budget dwarfs bf16's ~2^-9 rounding) and laid out channel-major
[CS, B, L+pads] on the host so each SBUF partition row is ONE contiguous
~32 KB HBM run: DMA descriptors amortize their fixed per-packet cost
(8 KB rows cap a ring near ~190 GB/s; 32 KB rows reach ~330). Loads
stream on the sync HWDGE ring, stores on the scalar ring — one direction
per ring, stores deferred one group-tile so ring heads never block.

Per-core compute: channels on partitions (128 at a time => 4 group
tiles), time on the free dim. x is host-padded with 3 zeros both sides
per batch segment, so out[m] = sum_t w_t * xp[m+t]. Work per 1024-col
span (PSUM pool tile = 2 banks):

  PE      : taps 0,1,3 — diagonal-weight bf16 matmuls (512-col slices)
            accumulating in PSUM; the odd-offset taps must avoid DVE
            (its 2x bf16 packing needs 4B-aligned reads)
  ScalarE : out_bf16 = psum + bias  (activation, per-partition bias)
  VectorE : out_bf16 += w2 * xp[m+2]  (scalar_tensor_tensor; +2 elems
            keeps reads 4B-aligned)

Diag weight matrices and per-channel consts are host-packed into three
small DMAs that lead the sync ring.
"""

import numpy as np

import concourse.bass as bass
import concourse.tile as tile
from concourse import bacc, mybir
from concourse.bass_utils import run_bass_kernel_spmd

B, C, L, K = 4, 4096, 4096, 4
PAD = K - 1
LOUT = L + PAD  # 4099
NCORES = 8
CS = C // NCORES  # 512 channels per core
NG = CS // 128  # 4 partition groups per core
WSEG = L + 2 * PAD + 2  # padded x segment width, 8B-aligned rows (4104)
OSEG = LOUT + 1  # stored segment width, 4B-aligned rows (4100)
F32 = mybir.dt.float32
BF16 = mybir.dt.bfloat16

PE_TAPS = (0, 1, 3)  # taps accumulated on PE via diag matmuls
DVE_TAP = 2  # tap fused into the final DVE pass (4B-aligned in bf16)
SPAN = 1024  # ACT/DVE span = one PSUM pool tile (2 banks)

_AF = mybir.ActivationFunctionType
_OP = mybir.AluOpType


def build_nc(b=B, cs=CS, n_bufs=2):
    ng = cs // 128
    nt = len(PE_TAPS)
    # spans per batch segment: 4x1024 + 4-col tail (col L+3 is garbage,
    # dropped on the host)
    spans = [(m0, SPAN) for m0 in range(0, L, SPAN)] + [(L, 4)]

    nc = bacc.Bacc("TRN2", target_bir_lowering=False, debug=False, num_devices=NCORES)
    x_d = nc.dram_tensor("x", [cs, b, WSEG], BF16, kind="ExternalInput").ap()
    dg_d = nc.dram_tensor("dg", [128, ng * nt * 128], BF16,
                          kind="ExternalInput").ap()
    ctw_d = nc.dram_tensor("ctw", [128, ng], BF16, kind="ExternalInput").ap()
    ctb_d = nc.dram_tensor("ctb", [128, ng], F32, kind="ExternalInput").ap()
    o_d = nc.dram_tensor("out", [cs, b, OSEG], BF16, kind="ExternalOutput").ap()

    with tile.TileContext(nc) as tc:
        with (
            tc.tile_pool(name="consts", bufs=1) as cpool,
            tc.tile_pool(name="xs", bufs=n_bufs) as xpool,
            tc.tile_pool(name="os", bufs=n_bufs) as opool,
            tc.tile_pool(name="ps", bufs=3, space="PSUM") as ppool,
            tc.tile_pool(name="pst", bufs=2, space="PSUM") as tpool,
        ):
            dgp = cpool.tile([128, ng * nt * 128], BF16, tag="dg")
            ctw = cpool.tile([128, ng], BF16, tag="cw")
            ctb = cpool.tile([128, ng], F32, tag="cb")

            def diag(g, j):
                o = (g * nt + j) * 128
                return dgp[:, o : o + 128]

            pending_stores = []  # deferred to keep the store ring head unblocked

            def flush_stores():
                for dst, src in pending_stores:
                    nc.scalar.dma_start(dst, src)
                pending_stores.clear()

            for g in range(ng):
                c0 = g * 128
                first, last = g == 0, g == ng - 1

                xt = xpool.tile([128, b, WSEG], BF16, tag="x")
                if first:
                    # consts lead (3 small DMAs), then segment 0 in two
                    # halves so compute ramps immediately
                    nc.sync.dma_start(ctw[:], ctw_d[:])
                    nc.sync.dma_start(ctb[:], ctb_d[:])
                    nc.sync.dma_start(dgp[:], dg_d[:])
                    h = WSEG // 2
                    nc.sync.dma_start(xt[:, 0, 0:h], x_d[c0 : c0 + 128, 0, 0:h])
                    nc.sync.dma_start(
                        xt[:, 0, h:WSEG], x_d[c0 : c0 + 128, 0, h:WSEG]
                    )
                    for bb in range(1, b):
                        nc.sync.dma_start(
                            xt[:, bb, :], x_d[c0 : c0 + 128, bb, :]
                        )
                else:
                    nc.sync.dma_start(xt[:, :, :], x_d[c0 : c0 + 128, :, :])
                ot = opool.tile([128, b, OSEG], BF16, tag="o")

                for bb in range(b):
                    for m0, fd in spans:
                        if fd == SPAN:
                            pt = ppool.tile([128, SPAN], F32, tag="p")
                        else:
                            pt = tpool.tile([128, 4], F32, tag="pt")
                        for s0 in range(0, fd, 512):
                            sw = min(512, fd - s0)
                            for j in range(nt):
                                t = PE_TAPS[j]
                                nc.tensor.matmul(
                                    pt[:, s0 : s0 + sw], lhsT=diag(g, j),
                                    rhs=xt[:, bb, m0 + s0 + t : m0 + s0 + t + sw],
                                    start=(j == 0), stop=(j == nt - 1),
                                )
                        # out = psum + bias (converts to bf16)
                        nc.scalar.activation(
                            ot[:, bb, m0 : m0 + fd], pt[:, 0:fd], _AF.Identity,
                            bias=ctb[:, g : g + 1], scale=1.0,
                        )
                        if bb == 0 and m0 == 0:
                            flush_stores()
                        # out += w2 * xp[m+2]
                        nc.vector.scalar_tensor_tensor(
                            out=ot[:, bb, m0 : m0 + fd],
                            in0=xt[:, bb, m0 + DVE_TAP : m0 + DVE_TAP + fd],
                            scalar=ctw[:, g : g + 1],
                            in1=ot[:, bb, m0 : m0 + fd],
                            op0=_OP.mult, op1=_OP.add,
                        )
                        if last and bb == b - 1:
                            # finest-grain stores on the very last segment
                            nc.scalar.dma_start(
                                o_d[c0 : c0 + 128, bb, m0 : m0 + fd],
                                ot[:, bb, m0 : m0 + fd],
                            )
                    if last and bb < b - 1:
                        nc.scalar.dma_start(
                            o_d[c0 : c0 + 128, bb, :], ot[:, bb, :]
                        )
                if not last:
                    pending_stores.append((o_d[c0 : c0 + 128, :, :], ot[:, :, :]))
            flush_stores()
    nc.compile()
    return nc


_cached_nc = None


def _get_nc():
    global _cached_nc
    if _cached_nc is None:
        _cached_nc = build_nc()
    return _cached_nc


def run(x, kernel, bias, trace=False, **kwargs):
    """Shard, run on 8 cores, gather. Returns (out, BassKernelResults)."""
    import ml_dtypes

    bf16 = ml_dtypes.bfloat16
    x_bf = np.asarray(x, dtype=np.float32).astype(bf16)  # [B, C, L]
    w = np.asarray(kernel, dtype=np.float32).reshape(K, C)
    bvec = np.asarray(bias, dtype=np.float32).reshape(C)

    w_bf = w.astype(bf16)
    nt = len(PE_TAPS)
    # channel-major, host-padded: xp[c, b, 3:L+3] = x[b, c, :]
    xp = np.zeros((C, B, WSEG), dtype=bf16)
    xp[:, :, PAD : PAD + L] = x_bf.transpose(1, 0, 2)

    in_maps = []
    for i in range(NCORES):
        sl = slice(i * CS, (i + 1) * CS)
        dg = np.zeros((NG * nt, 128, 128), dtype=bf16)
        ctw = np.zeros((128, NG), dtype=bf16)
        ctb = np.zeros((128, NG), dtype=np.float32)
        for g in range(NG):
            cg = slice(i * CS + g * 128, i * CS + (g + 1) * 128)
            for j, t in enumerate(PE_TAPS):
                np.fill_diagonal(dg[g * nt + j], w_bf[t, cg])
            ctw[:, g] = w_bf[DVE_TAP, cg]
            ctb[:, g] = bvec[cg]
        dg_pack = np.ascontiguousarray(
            dg.transpose(1, 0, 2).reshape(128, NG * nt * 128)
        )
        in_maps.append(
            {
                "x": np.ascontiguousarray(xp[sl]),
                "dg": dg_pack,
                "ctw": ctw,
                "ctb": ctb,
            }
        )

    nc = _get_nc()
    bkr = run_bass_kernel_spmd(
        nc, in_maps, core_ids=list(range(NCORES)), trace=trace, **kwargs
    )
    # [CS, B, OSEG] shards -> [B, C, LOUT] fp32
    out = np.concatenate(
        [r["out"][:, :, :LOUT].astype(np.float32) for r in bkr.results], axis=0
    ).transpose(1, 0, 2)
    return np.ascontiguousarray(out), bkr


def kernel(x, kernel, bias):
    import os

    prev = os.environ.get("BASS_NEVER_TRACE")
    os.environ["BASS_NEVER_TRACE"] = "1"  # keep the runner off the NTFF path
    try:
        out, _ = run(x, kernel, bias)
    finally:
        if prev is None:
            os.environ.pop("BASS_NEVER_TRACE", None)
        else:
            os.environ["BASS_NEVER_TRACE"] = prev
    return out
